# revision 18
# baseline (speedup 1.0000x reference)
"""LSTM encoder with EOS-freeze for Trainium2, data-parallel over batch on 8 cores.

Strategy (v3)
-------------
Inputs are one-hot, so x @ Wi is a row-gather of Wi done with indirect DMA on
device; rows for two consecutive steps are gathered into one [32, 2048] tile so
one LDWEIGHTS transposes both steps (two matmuls against stacked identities).

The recurrent h @ Wh runs with Wh as 64 fp16 [128,128] stationary tiles and h
chunks as the [128,16] moving operand. The LDWEIGHTS+MATMUL pair pitch is
~27 ns regardless of dtype (the weight port moves columns, not bytes), so the
h-block costs ~1.7 us/step; everything else is arranged to overlap it.

Gates live in FOUR persistent PSUM tiles per step parity (8 banks: g,i,f,o x
even,odd), each closed by its own `stop` matmul, so every activation fires as
soon as its own region finishes. Region order g,i,f,o leaves only
sigma(o) -> h on the post-block critical path. The x transposes for step s+2
are emitted at the end of step s, filling each step's tail-chain window.

Every activation is a single Sigmoid table set (no per-body table reloads):
the device state is halved (c' = c/2, h' = h/2, compensated in the host
weights and unshard), tanh(g) = 2*sg(2g)-1 via host-doubled g-columns and
tanh(c) = 2*sg(4c')-1 via the ACT scale port, with the affine corrections
folded into scalar_tensor_tensor multiplies at no extra op count.

The EOS freeze is handled without any per-step masking: sequences are
independent, so the kernel runs the unmasked recurrence and streams per-step
(c, h) snapshots to DRAM; the frozen value for sequence b is the snapshot at
its first-EOS step, selected during unshard.
"""

import numpy as np

try:
    import concourse  # noqa: F401
except ImportError:
    import sys

    sys.path.insert(0, "/opt/trn_rl_repo")

from contextlib import ExitStack

import concourse.bass as bass
import concourse.tile as tile
from concourse import bacc
from concourse import mybir
from concourse.bass import ds
from concourse.bass_utils import run_bass_kernel_spmd

dt = mybir.dt
Alu = mybir.AluOpType
Act = mybir.ActivationFunctionType

EOS_ID = 1
HID = 512
BATCH, SEQ, VOCAB = 128, 256, 1024
GATES = 4 * HID  # 2048
NCORES = 8
BLOC = BATCH // NCORES  # 16 sequences per core
BODY = 32  # steps per For_i iteration
PAIRS = BODY // 2  # x-gather/transpose granularity: 2 steps share one LDW

# Collect profiling info when True (set by test.py; adds trace overhead).
TRACE = False
LAST_RESULTS = None  # BassKernelResults of the last run, for test.py

_PROGRAM = None


def _build_program(seq=SEQ, body=BODY):
    pairs = body // 2
    nc = bacc.Bacc("TRN2", debug=False, detect_race_conditions=False)

    wi = nc.declare_dram_parameter("wi", [VOCAB, GATES], dt.float16, isOutput=False)
    ident = nc.declare_dram_parameter("ident", [32, 32], dt.float16, isOutput=False)
    wh = nc.declare_dram_parameter("wh", [128, 64 * 128], dt.float16, isOutput=False)
    tok2 = nc.declare_dram_parameter(
        "tok2", [32, seq // 2 + pairs], dt.int32, isOutput=False
    )
    c_traj = nc.declare_dram_parameter("c_traj", [seq * 128, 64], dt.float16, isOutput=True)
    h_traj = nc.declare_dram_parameter("h_traj", [seq * 128, 64], dt.float16, isOutput=True)

    with tile.TileContext(nc) as tc, ExitStack() as ctx:
        pool = lambda name, bufs, **kw: ctx.enter_context(
            tc.tile_pool(name=name, bufs=bufs, **kw)
        )
        whp = pool("whp", 1)
        tokp = pool("tokp", 1)
        stp = pool("stp", 1)
        hp = pool("hp", 1)
        cp = pool("cp", 1)
        zp = pool("zp", 1, space="PSUM")
        tgp = pool("tgp", 2)
        sip = pool("sip", 2)
        sfp = pool("sfp", 2)
        sop = pool("sop", 2)
        ap_ = pool("ap", 2)
        bp = pool("bp", 2)
        tp = pool("tp", 2)

        wh_sb = whp.tile([128, 64 * 128], dt.float16, name="wh_sb")
        nc.sync.dma_start(out=wh_sb[:], in_=wh[:, :])
        tok_cur = tokp.tile([32, pairs], dt.int32, name="tok_cur")
        nc.sync.dma_start(out=tok_cur[:], in_=tok2[:, 0:pairs])
        id_sb = tokp.tile([32, 32], dt.float16, name="id_sb")
        nc.sync.dma_start(out=id_sb[:], in_=ident[:, :])

        ST2 = [
            stp.tile([32, GATES], dt.float16, name=f"st{m}", tag=f"st{m}")
            for m in range(pairs)
        ]
        H = [hp.tile([128, 64], dt.float16, name=f"h{s}", tag=f"h{s}") for s in range(body)]
        C = [cp.tile([128, 64], dt.float16, name=f"c{s}", tag=f"c{s}") for s in range(body)]
        # Persistent per-parity gate region tiles: 4 regions x 2 parities = 8
        # PSUM banks. Region r of step s lives in Z[r][s % 2].
        Z = [
            [
                zp.tile([128, 64], dt.float32, name=f"z{r}{par}", tag=f"z{r}{par}")
                for par in range(2)
            ]
            for r in range(4)
        ]

        nc.gpsimd.memset(H[body - 1][:], 0.0)
        nc.gpsimd.memset(C[body - 1][:], 0.0)
        for m in range(pairs):
            # init shadow coverage; real values come from the indirect gathers
            nc.gpsimd.memset(ST2[m][:], 0.0)

        def gather_xp(m):
            # Gather 32 wi rows (2 steps x 16 sequences) into ST2[m][q, :] --
            # row-per-partition, the DGE-supported shape. tok_cur always holds
            # the token columns for the body being prefetched.
            nc.gpsimd.indirect_dma_start(
                out=ST2[m][:],
                out_offset=None,
                in_=wi[:, :],
                in_offset=bass.IndirectOffsetOnAxis(ap=tok_cur[:, m : m + 1], axis=0),
            )

        def xstep(s):
            # Transpose the gathered rows of step s (phase s%2 of pair slot
            # (s//2)%pairs) into its PSUM region tiles. start=True clears
            # has_written for the WHOLE bank, so only the first matmul
            # touching a region sets it. Emitted at the end of step s-2, so
            # these fill that step's tail-chain window on the PE queue.
            m = (s // 2) % pairs
            phase = s % 2
            par = s % 2
            for cch in range(16):
                r, j = cch // 4, cch % 4
                nc.tensor.matmul(
                    out=Z[r][par][:, 16 * j : 16 * j + 16],
                    lhsT=ST2[m][:, 128 * cch : 128 * cch + 128],
                    rhs=id_sb[:, 16 * phase : 16 * phase + 16],
                    start=(j == 0),
                    stop=False,
                    skip_group_check=True,
                )

        def hblock(s):
            par = s % 2
            hprev = H[(s - 1) % body]
            for r in range(4):
                for j in range(4):
                    for k in range(4):
                        t = (r * 4 + j) * 4 + k
                        nc.tensor.matmul(
                            out=Z[r][par][:, 16 * j : 16 * j + 16],
                            lhsT=wh_sb[:, t * 128 : t * 128 + 128],
                            rhs=hprev[:, 16 * k : 16 * k + 16],
                            start=False,
                            stop=(k == 3),
                            skip_group_check=True,
                        )

        def chain(iv2, s):
            # All activations are Sigmoid (one ACT table set, no per-body
            # reloads): the kernel state is halved, c' = c/2 and h' = h/2,
            # with the 2x folded into the host weights. tanh(g) = 2*sg(2g)-1
            # via host-doubled g-columns; tanh(c) = 2*sg(4c')-1 via the ACT
            # scale port. The (2x-1)/2 affines fold into the STT multiplies.
            par = s % 2
            TG = tgp.tile([128, 64], dt.float32, name="TG", tag="TG")
            nc.scalar.activation(out=TG[:], in_=Z[0][par][:], func=Act.Sigmoid)
            SI = sip.tile([128, 64], dt.float16, name="SI", tag="SI")
            nc.scalar.activation(out=SI[:], in_=Z[1][par][:], func=Act.Sigmoid)
            SF = sfp.tile([128, 64], dt.float16, name="SF", tag="SF")
            nc.scalar.activation(out=SF[:], in_=Z[2][par][:], func=Act.Sigmoid)
            SO = sop.tile([128, 64], dt.float16, name="SO", tag="SO")
            nc.scalar.activation(out=SO[:], in_=Z[3][par][:], func=Act.Sigmoid)
            A = ap_.tile([128, 64], dt.float16, name="A", tag="A")
            # A' = (sg(2g) - 0.5) * SI = SI * tanh(g) / 2
            nc.vector.scalar_tensor_tensor(
                out=A[:], in0=TG[:], scalar=0.5, in1=SI[:],
                op0=Alu.subtract, op1=Alu.mult,
            )
            B = bp.tile([128, 64], dt.float16, name="B", tag="B")
            nc.vector.tensor_tensor(out=B[:], in0=SF[:], in1=C[(s - 1) % body][:], op=Alu.mult)
            cs = C[s]
            nc.vector.tensor_tensor(out=cs[:], in0=A[:], in1=B[:], op=Alu.add)
            T = tp.tile([128, 64], dt.float32, name="T", tag="T")
            nc.scalar.activation(out=T[:], in_=cs[:], func=Act.Sigmoid, scale=4.0)
            hs = H[s]
            # h' = (sg(4c') - 0.5) * SO = SO * tanh(c) / 2
            nc.vector.scalar_tensor_tensor(
                out=hs[:], in0=T[:], scalar=0.5, in1=SO[:],
                op0=Alu.subtract, op1=Alu.mult,
            )

            nc.sync.dma_start(out=c_traj[ds((iv2 * 2 + s) * 128, 128), :], in_=cs[:])
            nc.sync.dma_start(out=h_traj[ds((iv2 * 2 + s) * 128, 128), :], in_=hs[:])

        # Preamble: gather all pair slots for body 0, transpose steps 0/1.
        for m in range(pairs):
            gather_xp(m)
        xstep(0)
        xstep(1)

        with tc.For_i(
            0,
            seq // 2,
            pairs,
            hint_engines=(mybir.EngineType.PE, mybir.EngineType.Activation),
            staggered_reset=True,
        ) as iv2:
            # Stage the NEXT body's token columns; in-loop gathers prefetch
            # for body i+1 while this body computes (tok2 is padded).
            nc.sync.dma_start(out=tok_cur[:], in_=tok2[:, ds(iv2 + pairs, pairs)])
            for p in range(pairs):
                s0, s1 = 2 * p, 2 * p + 1
                hblock(s0)
                chain(iv2, s0)
                # x for step s0+2 in this tail. Must be emitted AFTER
                # chain(s0): the start=True write invalidates the z that
                # step s0's activations read. (Letting the scheduler spill
                # these into hblock(s1) measures faster than corralling them
                # into the odd tail, where they FIFO-block the next block.)
                xstep((s0 + 2) % body)
                hblock(s1)
                gather_xp(p)
                chain(iv2, s1)
                xstep((s1 + 2) % body)

    nc.finalize()
    return nc


def _get_program():
    global _PROGRAM
    if _PROGRAM is None:
        _PROGRAM = _build_program()
    return _PROGRAM


def _prep_host(inputs, Wi, Wh, b):
    tokens = np.argmax(inputs, axis=-1).astype(np.int32)  # [B, T]
    eos = inputs[:, :, EOS_ID] > 0.5
    any_eos = eos.any(axis=1)
    t_star = np.where(any_eos, eos.argmax(axis=1), SEQ - 1).astype(np.int64)

    # Gate reorder (g, i, f, o): tanh region first, sigma(o) last so only the
    # o-path trails the matmul block.
    perm = np.concatenate(
        [np.arange(1024, 1536), np.arange(0, 512), np.arange(512, 1024), np.arange(1536, 2048)]
    )
    Wi_re = (Wi.astype(np.float32) + b.astype(np.float32)[None, :])[:, perm]
    Wh_re = Wh.astype(np.float32)[:, perm]
    # Device state is halved (h' = h/2): fold the 2x into Wh. The g-region
    # (first 512 permuted columns) gets another 2x so sigma(2g) gives tanh.
    Wh_re *= 2.0
    Wi_re[:, 0:512] *= 2.0
    Wh_re[:, 0:512] *= 2.0

    Wi_dev = np.ascontiguousarray(Wi_re).astype(np.float16)
    # wh[kr, ((r*4+j)*4+k)*128 + p] = Wh_re[128k+kr, 512r+128j+p]
    W5 = Wh_re.reshape(4, 128, 4, 4, 128)  # [k, kr, r, j, p]
    Wh_dev = np.ascontiguousarray(
        W5.transpose(1, 2, 3, 0, 4).reshape(128, 64 * 128)
    ).astype(np.float16)
    return tokens, t_star, Wi_dev, Wh_dev


def kernel(inputs, Wi, Wh, b):
    global LAST_RESULTS
    inputs = np.asarray(inputs)
    Wi = np.asarray(Wi)
    Wh = np.asarray(Wh)
    b = np.asarray(b)

    tokens, t_star, Wi_dev, Wh_dev = _prep_host(inputs, Wi, Wh, b)

    in_maps = []
    for n in range(NCORES):
        tokc = tokens[BLOC * n : BLOC * (n + 1)]  # [16, 256]
        tk = tokc.reshape(BLOC, SEQ // 2, 2)
        tok2 = np.concatenate([tk[:, :, 0], tk[:, :, 1]], axis=0)  # [32, 128]
        tok2 = np.concatenate([tok2, np.zeros((32, PAIRS), np.int32)], axis=1)
        in_maps.append(
            {
                "wi": Wi_dev,
                "wh": Wh_dev,
                "tok2": np.ascontiguousarray(tok2),
                "ident": np.eye(32, dtype=np.float16),
            }
        )

    nc = _get_program()
    res = run_bass_kernel_spmd(nc, in_maps, list(range(NCORES)), trace=TRACE)
    LAST_RESULTS = res

    c_out = np.zeros((BATCH, HID), np.float32)
    h_out = np.zeros((BATCH, HID), np.float32)
    for n in range(NCORES):
        # Device snapshots hold the halved state c' = c/2, h' = h/2.
        ct = res.results[n]["c_traj"].reshape(SEQ, 128, 64).astype(np.float32) * 2.0
        ht = res.results[n]["h_traj"].reshape(SEQ, 128, 64).astype(np.float32) * 2.0
        for bl in range(BLOC):
            g = BLOC * n + bl
            t = int(t_star[g])
            c_out[g] = ct[t][:, bl::BLOC].T.reshape(HID)
            h_out[g] = ht[t][:, bl::BLOC].T.reshape(HID)
    return (c_out, h_out)
